# revision 7
# baseline (speedup 1.0000x reference)
"""Binarized ResNet BasicBlock (conv1 3x3/s2 + BN + sign, conv2 3x3 + BN,
1x1/s2 shortcut conv + BN, add, sign) as a Bass/Tile kernel on 8 TRN2 cores.

Strategy:
- Data-parallel over batch: 16 images per core, weights/BN replicated.
- All matmuls in bf16. Binarized weights are exactly +-1 in bf16. x is split
  into 3 bf16 terms (hi/mid/lo) that sum exactly to the fp32 value, so
  conv1/shortcut accumulate the exact fp32 products in fp32 PSUM. conv2's
  inputs are +-1 (exact); its accumulation is exact integer arithmetic.
- sign(clip(bn(z))) == sign(bn(z)): fused into one Sign activation with
  per-channel scale/bias.
"""

import numpy as np
import ml_dtypes
from contextlib import ExitStack

import concourse.bass as bass
import concourse.tile as tile
from concourse import mybir, bacc
from concourse.bass_utils import run_bass_kernel_spmd

bf16 = ml_dtypes.bfloat16
F32 = mybir.dt.float32
BF = mybir.dt.bfloat16
SIGN = mybir.ActivationFunctionType.Sign
IDENT = mybir.ActivationFunctionType.Identity

N_CORES = 8
B, CIN, COUT, H = 128, 256, 512, 28
OH = 14                      # output spatial
BPC = B // N_CORES           # images per core
G = 2                        # images per matmul group
NG = BPC // G                # groups per core
NPG = G * OH * OH            # 392 psum columns per group
NCT = COUT // 128            # cout tiles (4)
NCI1 = CIN // 128            # cin tiles for conv1/shortcut (2)
NCI2 = COUT // 128           # cin tiles for conv2 (4)
EPS = np.float32(1e-5)

_prog_cache = {}


def _build_program():
    nc = bacc.Bacc("TRN2", debug=False)

    xp = [nc.dram_tensor(f"xp{ci}", [128, BPC, 29, 29], F32,
                         kind="ExternalInput").ap() for ci in range(NCI1)]
    w1 = nc.dram_tensor("w1t", [128, 9 * NCI1, COUT], BF, kind="ExternalInput").ap()
    w2 = nc.dram_tensor("w2t", [128, 9 * NCI2, COUT], BF, kind="ExternalInput").ap()
    wsc = nc.dram_tensor("wsct", [128, NCI1, COUT], BF, kind="ExternalInput").ap()
    bnc = nc.dram_tensor("bnc", [128, 5, NCT], F32, kind="ExternalInput").ap()
    y = nc.dram_tensor("y", [128, NCT, BPC, OH * OH], F32,
                       kind="ExternalOutput").ap()

    with tile.TileContext(nc) as tc, ExitStack() as ctx:
        consts = ctx.enter_context(tc.tile_pool(name="consts", bufs=1))
        xst = ctx.enter_context(tc.tile_pool(name="xst", bufs=4))
        spl = ctx.enter_context(tc.tile_pool(name="spl", bufs=12))
        a1p = ctx.enter_context(tc.tile_pool(name="a1p", bufs=8))
        yp = ctx.enter_context(tc.tile_pool(name="yp", bufs=2))
        up = ctx.enter_context(tc.tile_pool(name="up", bufs=6))
        pA = ctx.enter_context(tc.tile_pool(name="pA", bufs=4, space="PSUM"))
        pS = ctx.enter_context(tc.tile_pool(name="pS", bufs=4, space="PSUM"))

        w1_sb = consts.tile([128, 9 * NCI1, COUT], BF)
        nc.sync.dma_start(w1_sb[:], w1[:])
        w2_sb = consts.tile([128, 9 * NCI2, COUT], BF)
        nc.sync.dma_start(w2_sb[:], w2[:])
        wsc_sb = consts.tile([128, NCI1, COUT], BF)
        nc.sync.dma_start(wsc_sb[:], wsc[:])
        bnc_sb = consts.tile([128, 5, NCT], F32)
        nc.sync.dma_start(bnc_sb[:], bnc[:])

        def bn_ap(i, c):
            return bnc_sb[:, i, c:c + 1]

        for g in range(NG):
            bsl = slice(g * G, (g + 1) * G)
            # ---- load x and split into 3 exact bf16 terms (in-place resid) --
            parts = []  # parts[ci] = (hi, mid, lo)
            for ci in range(NCI1):
                xt = xst.tile([128, G, 29, 29], F32, tag="xst",
                              name=f"x_{g}_{ci}")
                nc.sync.dma_start(xt[:], xp[ci][:, bsl])
                hi = spl.tile([128, G, 29, 29], BF, tag="spl", name=f"hi_{g}_{ci}")
                nc.scalar.copy(hi[:], xt[:])
                nc.vector.tensor_sub(xt[:], xt[:], hi[:])
                mid = spl.tile([128, G, 29, 29], BF, tag="spl", name=f"mid_{g}_{ci}")
                nc.scalar.copy(mid[:], xt[:])
                nc.vector.tensor_sub(xt[:], xt[:], mid[:])
                lo = spl.tile([128, G, 29, 29], BF, tag="spl", name=f"lo_{g}_{ci}")
                nc.vector.tensor_copy(lo[:], xt[:])
                parts.append((hi, mid, lo))

            # ---- conv1: 9 taps x 2 cin tiles x 3 split terms ----
            p1 = []
            for c in range(NCT):
                pt = pA.tile([128, NPG], F32, tag="pA", name=f"p1_{g}_{c}")
                # split term outermost so consecutive matmuls never share a
                # stationary weight tile (same-weight back-to-back serializes
                # LDWEIGHTS; alternating pipelines at ~N/2.4 ns)
                idx, last = 0, NCI1 * 9 * 3 - 1
                for s in range(3):
                    for ci in range(NCI1):
                        for t in range(9):
                            kh, kw = divmod(t, 3)
                            w_ap = w1_sb[:, t * NCI1 + ci, c * 128:(c + 1) * 128]
                            rhs = parts[ci][s][:, :, kh:kh + 27:2, kw:kw + 27:2]
                            nc.tensor.matmul(pt[:], w_ap, rhs,
                                             start=(idx == 0), stop=(idx == last))
                            idx += 1
                p1.append(pt)

            # ---- shortcut: 1x1 stride-2 conv (center elements) ----
            psc = []
            for c in range(NCT):
                pt = pS.tile([128, NPG], F32, tag="pS", name=f"psc_{g}_{c}")
                idx, last = 0, NCI1 * 3 - 1
                for s in range(3):
                    for ci in range(NCI1):
                        w_ap = wsc_sb[:, ci, c * 128:(c + 1) * 128]
                        rhs = parts[ci][s][:, :, 1:28:2, 1:28:2]
                        nc.tensor.matmul(pt[:], w_ap, rhs,
                                         start=(idx == 0), stop=(idx == last))
                        idx += 1
                psc.append(pt)

            # ---- a1 = sign(bn1(conv1)), zero-padded borders, bf16 ----
            a1 = []
            for c in range(NCT):
                at = a1p.tile([128, G, 16, 16], BF, tag="a1", name=f"a1_{g}_{c}")
                nc.gpsimd.memset(at[:], 0.0)
                nc.scalar.activation(
                    at[:, :, 1:15, 1:15],
                    p1[c][:].rearrange("p (b h w) -> p b h w", b=G, h=OH, w=OH),
                    SIGN, bias=bn_ap(1, c), scale=bn_ap(0, c))
                a1.append(at)

            # ---- u = scalesc*psc + (shift2+shiftsc)  (frees pS banks) ----
            us = []
            for c in range(NCT):
                ut = up.tile([128, NPG], F32, tag="u", name=f"u_{g}_{c}")
                nc.scalar.activation(ut[:], psc[c][:], IDENT,
                                     bias=bn_ap(3, c), scale=bn_ap(4, c))
                us.append(ut)

            # ---- conv2: 9 taps x 4 cin tiles ----
            p2 = []
            for c in range(NCT):
                pt = pA.tile([128, NPG], F32, tag="pA", name=f"p2_{g}_{c}")
                idx, last = 0, NCI2 * 9 - 1
                for ci in range(NCI2):
                    for t in range(9):
                        kh, kw = divmod(t, 3)
                        w_ap = w2_sb[:, t * NCI2 + ci, c * 128:(c + 1) * 128]
                        rhs = a1[ci][:, :, kh:kh + 14, kw:kw + 14]
                        nc.tensor.matmul(pt[:], w_ap, rhs,
                                         start=(idx == 0), stop=(idx == last))
                        idx += 1
                p2.append(pt)

            # ---- y = sign(scale2*p2 + u) ----
            yt = yp.tile([128, NCT, NPG], F32, tag="y", name=f"y_{g}")
            for c in range(NCT):
                vt = up.tile([128, NPG], F32, tag="v", bufs=3, name=f"v_{g}_{c}")
                nc.vector.scalar_tensor_tensor(
                    vt[:], p2[c][:], bn_ap(2, c), us[c][:],
                    op0=mybir.AluOpType.mult, op1=mybir.AluOpType.add)
                nc.scalar.activation(yt[:, c, :], vt[:], SIGN)
            nc.sync.dma_start(
                y[:, :, bsl].rearrange("p c b x -> p c (b x)"), yt[:])

    nc.compile()
    return nc


def _prep_consts(w1, w2, wsc, g1, b1, m1, v1, g2, b2, m2, v2, gsc, bsc, msc, vsc):
    def sgn_w(w):
        return np.where(w >= 0, np.float32(1.0), np.float32(-1.0)).astype(bf16)

    # lhsT layouts: [cin_part(128), tap*NCI+ci, cout]
    w1s = sgn_w(w1)  # [COUT, CIN, 3, 3]
    a1w = np.empty((128, 9 * NCI1, COUT), bf16)
    for t in range(9):
        kh, kw = divmod(t, 3)
        for ci in range(NCI1):
            a1w[:, t * NCI1 + ci, :] = w1s[:, ci * 128:(ci + 1) * 128, kh, kw].T
    w2s = sgn_w(w2)
    a2w = np.empty((128, 9 * NCI2, COUT), bf16)
    for t in range(9):
        kh, kw = divmod(t, 3)
        for ci in range(NCI2):
            a2w[:, t * NCI2 + ci, :] = w2s[:, ci * 128:(ci + 1) * 128, kh, kw].T
    wscs = sgn_w(wsc)
    asw = np.empty((128, NCI1, COUT), bf16)
    for ci in range(NCI1):
        asw[:, ci, :] = wscs[:, ci * 128:(ci + 1) * 128, 0, 0].T

    def bn_affine(g, b, m, v):
        scale = (g / np.sqrt(v + EPS)).astype(np.float32)
        shift = (b - m * g / np.sqrt(v + EPS)).astype(np.float32)
        return scale, shift

    sc1, sh1 = bn_affine(g1, b1, m1, v1)
    sc2, sh2 = bn_affine(g2, b2, m2, v2)
    scs, shs = bn_affine(gsc, bsc, msc, vsc)
    bnc = np.empty((128, 5, NCT), np.float32)
    for c in range(NCT):
        cs = slice(c * 128, (c + 1) * 128)
        bnc[:, 0, c] = sc1[cs]
        bnc[:, 1, c] = sh1[cs]
        bnc[:, 2, c] = sc2[cs]
        bnc[:, 3, c] = (sh2 + shs)[cs]
        bnc[:, 4, c] = scs[cs]
    return a1w, a2w, asw, bnc


def kernel(x, w1, g1, b1, m1, v1, w2, g2, b2, m2, v2, wsc, gsc, bsc, msc, vsc,
           _trace=False):
    x = np.ascontiguousarray(x, np.float32)
    a1w, a2w, asw, bnc = _prep_consts(
        np.asarray(w1, np.float32), np.asarray(w2, np.float32),
        np.asarray(wsc, np.float32),
        *[np.asarray(t, np.float32) for t in (g1, b1, m1, v1)],
        *[np.asarray(t, np.float32) for t in (g2, b2, m2, v2)],
        *[np.asarray(t, np.float32) for t in (gsc, bsc, msc, vsc)])

    # padded, channel-major x: xpad[c, b, 1+h, 1+w]
    xpad = np.zeros((CIN, B, H + 1, H + 1), np.float32)
    xpad[:, :, 1:, 1:] = x.transpose(1, 0, 2, 3)

    if "nc" not in _prog_cache:
        _prog_cache["nc"] = _build_program()
    nc = _prog_cache["nc"]

    in_maps = []
    for k in range(N_CORES):
        m = {"w1t": a1w, "w2t": a2w, "wsct": asw, "bnc": bnc}
        for ci in range(NCI1):
            m[f"xp{ci}"] = np.ascontiguousarray(
                xpad[ci * 128:(ci + 1) * 128, k * BPC:(k + 1) * BPC])
        in_maps.append(m)

    res = run_bass_kernel_spmd(nc, in_maps, core_ids=list(range(N_CORES)),
                               trace=_trace)

    # y dram: [128, NCT, BPC, 196] per core -> [B, COUT, 14, 14]
    out = np.empty((B, COUT, OH, OH), np.float32)
    for k in range(N_CORES):
        yk = res.results[k]["y"]  # [128, 4, 16, 196]
        out[k * BPC:(k + 1) * BPC] = (
            yk.transpose(2, 1, 0, 3).reshape(BPC, COUT, OH, OH))
    if _trace:
        kernel.last_results = res
    return out


# revision 8
# speedup vs baseline: 1.3008x; 1.3008x over previous
"""Binarized ResNet BasicBlock (conv1 3x3/s2 + BN + sign, conv2 3x3 + BN,
1x1/s2 shortcut conv + BN, add, sign) as a Bass/Tile kernel on 8 TRN2 cores.

Strategy:
- Data-parallel over batch: 16 images per core, weights/BN params replicated.
- Binarized weights are exactly +-1 in fp16/bf16. x is split into 2 fp16
  terms (hi = fp16(x), lo = fp16(x - hi)) whose products with +-1 weights
  are exact (PE handles fp16 subnormals exactly; verified), so conv1 and the
  shortcut accumulate x to ~2^-22 relative in fp32 PSUM. conv2's inputs are
  +-1 bf16 (exact); its accumulation is exact integer arithmetic in fp32.
- sign(clip(bn(z))) == sign(bn(z)): fused into one Sign activation with
  per-channel scale/bias APs.
- Consecutive matmuls never reuse a stationary weight tile (same-weight
  back-to-back serializes LDWEIGHTS; rotating weights pipelines it away).
"""

import numpy as np
import ml_dtypes
from contextlib import ExitStack

import concourse.bass as bass
import concourse.tile as tile
from concourse import mybir, bacc
from concourse.bass_utils import run_bass_kernel_spmd

bf16 = ml_dtypes.bfloat16
F32 = mybir.dt.float32
BF = mybir.dt.bfloat16
F16 = mybir.dt.float16
SIGN = mybir.ActivationFunctionType.Sign
IDENT = mybir.ActivationFunctionType.Identity

N_CORES = 8
B, CIN, COUT, H = 128, 256, 512, 28
OH = 14                      # output spatial
BPC = B // N_CORES           # images per core
G = 2                        # images per matmul group
NG = BPC // G                # groups per core
NPG = G * OH * OH            # 392 psum columns per group
NCT = COUT // 128            # cout tiles (4)
NCI1 = CIN // 128            # cin tiles for conv1/shortcut (2)
NCI2 = COUT // 128           # cin tiles for conv2 (4)
NSPL = 2                     # split terms for x (fp16 hi/lo)
EPS = np.float32(1e-5)

_prog_cache = {}


def _build_program():
    nc = bacc.Bacc("TRN2", debug=False)

    xp = [nc.dram_tensor(f"xp{ci}", [128, BPC, 29, 29], F32,
                         kind="ExternalInput").ap() for ci in range(NCI1)]
    w1 = nc.dram_tensor("w1t", [128, 9 * NCI1, COUT], F16, kind="ExternalInput").ap()
    w2 = nc.dram_tensor("w2t", [128, 9 * NCI2, COUT], BF, kind="ExternalInput").ap()
    wsc = nc.dram_tensor("wsct", [128, NCI1, COUT], F16, kind="ExternalInput").ap()
    bnc = nc.dram_tensor("bnc", [128, 5, NCT], F32, kind="ExternalInput").ap()
    y = nc.dram_tensor("y", [128, NCT, BPC, OH * OH], F32,
                       kind="ExternalOutput").ap()

    with tile.TileContext(nc) as tc, ExitStack() as ctx:
        consts = ctx.enter_context(tc.tile_pool(name="consts", bufs=1))
        xst = ctx.enter_context(tc.tile_pool(name="xst", bufs=4))
        spl = ctx.enter_context(tc.tile_pool(name="spl", bufs=8))
        a1p = ctx.enter_context(tc.tile_pool(name="a1p", bufs=8))
        yp = ctx.enter_context(tc.tile_pool(name="yp", bufs=2))
        up = ctx.enter_context(tc.tile_pool(name="up", bufs=6))
        pA = ctx.enter_context(tc.tile_pool(name="pA", bufs=4, space="PSUM"))
        pS = ctx.enter_context(tc.tile_pool(name="pS", bufs=4, space="PSUM"))

        # w1/wsc/bnc load first (needed by group 0); w2's DMA is emitted after
        # group 0's split section so it doesn't block startup.
        w1_sb = consts.tile([128, 9 * NCI1, COUT], F16)
        wsc_sb = consts.tile([128, NCI1, COUT], F16)
        bnc_sb = consts.tile([128, 5, NCT], F32)
        w2_sb = consts.tile([128, 9 * NCI2, COUT], BF)

        def bn_ap(i, c):
            return bnc_sb[:, i, c:c + 1]

        for g in range(NG):
            bsl = slice(g * G, (g + 1) * G)
            # ---- load x and split into 2 exact fp16 terms (in-place resid) --
            parts = []  # parts[ci] = (hi, lo)
            for ci in range(NCI1):
                xt = xst.tile([128, G, 29, 29], F32, tag="xst",
                              name=f"x_{g}_{ci}")
                nc.sync.dma_start(xt[:], xp[ci][:, bsl])
                if g == 0 and ci == 0:
                    nc.sync.dma_start(w1_sb[:], w1[:])
                    nc.sync.dma_start(wsc_sb[:], wsc[:])
                    nc.sync.dma_start(bnc_sb[:], bnc[:])
                hi = spl.tile([128, G, 29, 29], F16, tag="spl", name=f"hi_{g}_{ci}")
                nc.scalar.copy(hi[:], xt[:])
                nc.vector.tensor_sub(xt[:], xt[:], hi[:])
                lo = spl.tile([128, G, 29, 29], F16, tag="spl", name=f"lo_{g}_{ci}")
                nc.vector.tensor_copy(lo[:], xt[:])
                parts.append((hi, lo))
            if g == 0:
                nc.sync.dma_start(w2_sb[:], w2[:])

            # ---- conv1: 9 taps x 2 cin tiles x 2 split terms ----
            p1 = []
            for c in range(NCT):
                pt = pA.tile([128, NPG], F32, tag="pA", name=f"p1_{g}_{c}")
                idx, last = 0, NCI1 * 9 * NSPL - 1
                for s in range(NSPL):
                    for ci in range(NCI1):
                        for t in range(9):
                            kh, kw = divmod(t, 3)
                            w_ap = w1_sb[:, t * NCI1 + ci, c * 128:(c + 1) * 128]
                            rhs = parts[ci][s][:, :, kh:kh + 27:2, kw:kw + 27:2]
                            nc.tensor.matmul(pt[:], w_ap, rhs,
                                             start=(idx == 0), stop=(idx == last))
                            idx += 1
                p1.append(pt)

            # ---- shortcut: 1x1 stride-2 conv (center elements) ----
            psc = []
            for c in range(NCT):
                pt = pS.tile([128, NPG], F32, tag="pS", name=f"psc_{g}_{c}")
                idx, last = 0, NCI1 * NSPL - 1
                for s in range(NSPL):
                    for ci in range(NCI1):
                        w_ap = wsc_sb[:, ci, c * 128:(c + 1) * 128]
                        rhs = parts[ci][s][:, :, 1:28:2, 1:28:2]
                        nc.tensor.matmul(pt[:], w_ap, rhs,
                                         start=(idx == 0), stop=(idx == last))
                        idx += 1
                psc.append(pt)

            # ---- a1 = sign(bn1(conv1)), zero-padded borders, bf16 ----
            a1 = []
            for c in range(NCT):
                at = a1p.tile([128, G, 16, 16], BF, tag="a1", name=f"a1_{g}_{c}")
                nc.gpsimd.memset(at[:], 0.0)
                nc.scalar.activation(
                    at[:, :, 1:15, 1:15],
                    p1[c][:].rearrange("p (b h w) -> p b h w", b=G, h=OH, w=OH),
                    SIGN, bias=bn_ap(1, c), scale=bn_ap(0, c))
                a1.append(at)

            # ---- u = scalesc*psc + (shift2+shiftsc)  (frees pS banks) ----
            us = []
            for c in range(NCT):
                ut = up.tile([128, NPG], F32, tag="u", name=f"u_{g}_{c}")
                nc.scalar.activation(ut[:], psc[c][:], IDENT,
                                     bias=bn_ap(3, c), scale=bn_ap(4, c))
                us.append(ut)

            # ---- conv2: 9 taps x 4 cin tiles ----
            p2 = []
            for c in range(NCT):
                pt = pA.tile([128, NPG], F32, tag="pA", name=f"p2_{g}_{c}")
                idx, last = 0, NCI2 * 9 - 1
                for ci in range(NCI2):
                    for t in range(9):
                        kh, kw = divmod(t, 3)
                        w_ap = w2_sb[:, t * NCI2 + ci, c * 128:(c + 1) * 128]
                        rhs = a1[ci][:, :, kh:kh + 14, kw:kw + 14]
                        nc.tensor.matmul(pt[:], w_ap, rhs,
                                         start=(idx == 0), stop=(idx == last))
                        idx += 1
                p2.append(pt)

            # ---- y = sign(scale2*p2 + u) ----
            yt = yp.tile([128, NCT, NPG], F32, tag="y", name=f"y_{g}")
            for c in range(NCT):
                vt = up.tile([128, NPG], F32, tag="v", bufs=3, name=f"v_{g}_{c}")
                nc.vector.scalar_tensor_tensor(
                    vt[:], p2[c][:], bn_ap(2, c), us[c][:],
                    op0=mybir.AluOpType.mult, op1=mybir.AluOpType.add)
                nc.scalar.activation(yt[:, c, :], vt[:], SIGN)
            nc.sync.dma_start(
                y[:, :, bsl].rearrange("p c b x -> p c (b x)"), yt[:])

    nc.compile()
    return nc


def _prep_consts(w1, w2, wsc, g1, b1, m1, v1, g2, b2, m2, v2, gsc, bsc, msc, vsc):
    def sgn_w(w, dt):
        return np.where(w >= 0, np.float32(1.0), np.float32(-1.0)).astype(dt)

    # lhsT layouts: [cin_part(128), tap*NCI+ci, cout]
    w1s = sgn_w(w1, np.float16)  # [COUT, CIN, 3, 3]
    a1w = np.empty((128, 9 * NCI1, COUT), np.float16)
    for t in range(9):
        kh, kw = divmod(t, 3)
        for ci in range(NCI1):
            a1w[:, t * NCI1 + ci, :] = w1s[:, ci * 128:(ci + 1) * 128, kh, kw].T
    w2s = sgn_w(w2, bf16)
    a2w = np.empty((128, 9 * NCI2, COUT), bf16)
    for t in range(9):
        kh, kw = divmod(t, 3)
        for ci in range(NCI2):
            a2w[:, t * NCI2 + ci, :] = w2s[:, ci * 128:(ci + 1) * 128, kh, kw].T
    wscs = sgn_w(wsc, np.float16)
    asw = np.empty((128, NCI1, COUT), np.float16)
    for ci in range(NCI1):
        asw[:, ci, :] = wscs[:, ci * 128:(ci + 1) * 128, 0, 0].T

    def bn_affine(g, b, m, v):
        scale = (g / np.sqrt(v + EPS)).astype(np.float32)
        shift = (b - m * g / np.sqrt(v + EPS)).astype(np.float32)
        return scale, shift

    sc1, sh1 = bn_affine(g1, b1, m1, v1)
    sc2, sh2 = bn_affine(g2, b2, m2, v2)
    scs, shs = bn_affine(gsc, bsc, msc, vsc)
    bnc = np.empty((128, 5, NCT), np.float32)
    for c in range(NCT):
        cs = slice(c * 128, (c + 1) * 128)
        bnc[:, 0, c] = sc1[cs]
        bnc[:, 1, c] = sh1[cs]
        bnc[:, 2, c] = sc2[cs]
        bnc[:, 3, c] = (sh2 + shs)[cs]
        bnc[:, 4, c] = scs[cs]
    return a1w, a2w, asw, bnc


def kernel(x, w1, g1, b1, m1, v1, w2, g2, b2, m2, v2, wsc, gsc, bsc, msc, vsc,
           _trace=False):
    x = np.ascontiguousarray(x, np.float32)
    a1w, a2w, asw, bnc = _prep_consts(
        np.asarray(w1, np.float32), np.asarray(w2, np.float32),
        np.asarray(wsc, np.float32),
        *[np.asarray(t, np.float32) for t in (g1, b1, m1, v1)],
        *[np.asarray(t, np.float32) for t in (g2, b2, m2, v2)],
        *[np.asarray(t, np.float32) for t in (gsc, bsc, msc, vsc)])

    # padded, channel-major x: xpad[c, b, 1+h, 1+w]
    xpad = np.zeros((CIN, B, H + 1, H + 1), np.float32)
    xpad[:, :, 1:, 1:] = x.transpose(1, 0, 2, 3)

    if "nc" not in _prog_cache:
        _prog_cache["nc"] = _build_program()
    nc = _prog_cache["nc"]

    in_maps = []
    for k in range(N_CORES):
        m = {"w1t": a1w, "w2t": a2w, "wsct": asw, "bnc": bnc}
        for ci in range(NCI1):
            m[f"xp{ci}"] = np.ascontiguousarray(
                xpad[ci * 128:(ci + 1) * 128, k * BPC:(k + 1) * BPC])
        in_maps.append(m)

    res = run_bass_kernel_spmd(nc, in_maps, core_ids=list(range(N_CORES)),
                               trace=_trace)

    # y dram: [128, NCT, BPC, 196] per core -> [B, COUT, 14, 14]
    out = np.empty((B, COUT, OH, OH), np.float32)
    for k in range(N_CORES):
        yk = res.results[k]["y"]  # [128, 4, 16, 196]
        out[k * BPC:(k + 1) * BPC] = (
            yk.transpose(2, 1, 0, 3).reshape(BPC, COUT, OH, OH))
    if _trace:
        kernel.last_results = res
    return out


# revision 12
# speedup vs baseline: 1.3686x; 1.0521x over previous
"""Binarized ResNet BasicBlock (conv1 3x3/s2 + BN + sign, conv2 3x3 + BN,
1x1/s2 shortcut conv + BN, add, sign) as a Bass/Tile kernel on 8 TRN2 cores.

Strategy:
- Data-parallel over batch: 16 images per core, weights/BN params replicated.
- Binarized weights are exactly +-1 in fp16/bf16. x is split into 2 fp16
  terms (hi = fp16(x), lo = fp16(x - hi)) whose products with +-1 weights
  are exact (PE handles fp16 subnormals exactly; verified on HW), so conv1
  and the shortcut accumulate x to ~2^-22 relative in fp32 PSUM. conv2's
  inputs are +-1 bf16 (exact); its accumulation is exact integer arithmetic.
- sign(clip(bn(z))) == sign(bn(z)): fused into one Sign activation with
  per-channel scale/bias APs.
- Consecutive matmuls never reuse a stationary weight tile (same-weight
  back-to-back serializes LDWEIGHTS; rotating weights pipelines it away).
- Conv taps stream fully-contiguous rhs and land at tap-dependent PSUM
  offsets (per-element has_written gives overwrite-then-accumulate), instead
  of strided 28-byte windows: x is packed as stride-2 parity planes so every
  conv1 tap reads one contiguous span; conv2 streams whole unpadded a1
  tiles into a 16x16 PSUM window with garbage borders.
"""

import numpy as np
import ml_dtypes
from contextlib import ExitStack

import concourse.bass as bass
import concourse.tile as tile
from concourse import mybir, bacc
from concourse.bass_utils import run_bass_kernel_spmd

bf16 = ml_dtypes.bfloat16
F32 = mybir.dt.float32
BF = mybir.dt.bfloat16
F16 = mybir.dt.float16
SIGN = mybir.ActivationFunctionType.Sign
IDENT = mybir.ActivationFunctionType.Identity

N_CORES = 8
B, CIN, COUT, H = 128, 256, 512, 28
OH = 14                      # output spatial
BPC = B // N_CORES           # images per core
G = 2                        # images per matmul group
NG = BPC // G                # groups per core
NPG = G * OH * OH            # 392 valid pixels per group
NCT = COUT // 128            # cout tiles (4)
NCI1 = CIN // 128            # cin tiles for conv1/shortcut (2)
NCI2 = COUT // 128           # cin tiles for conv2 (4)
NSPL = 2                     # split terms for x (fp16 hi/lo)
EPS = np.float32(1e-5)

# parity-plane packing of the 29x29 zero-padded input (pad at index 0):
# plane (ph, pw) = xpad[2i+ph, 2j+pw]; heights/widths 15 or 14.
PL_H = {0: 15, 1: 14}
PL_W = {0: 15, 1: 14}
PL_OFF = {(0, 0): 0, (0, 1): 225, (1, 0): 435, (1, 1): 645}

_prog_cache = {}


def _build_program():
    nc = bacc.Bacc("TRN2", debug=False)

    xp = [nc.dram_tensor(f"xp{ci}", [128, BPC, 841], F32,
                         kind="ExternalInput").ap() for ci in range(NCI1)]
    w1 = nc.dram_tensor("w1t", [128, 9 * NCI1, COUT], F16, kind="ExternalInput").ap()
    w2 = nc.dram_tensor("w2t", [128, 9 * NCI2, COUT], BF, kind="ExternalInput").ap()
    wsc = nc.dram_tensor("wsct", [128, NCI1, COUT], F16, kind="ExternalInput").ap()
    bnc = nc.dram_tensor("bnc", [128, 5, NCT], F32, kind="ExternalInput").ap()
    y = nc.dram_tensor("y", [128, NCT, BPC, OH * OH], F32,
                       kind="ExternalOutput").ap()

    with tile.TileContext(nc) as tc, ExitStack() as ctx:
        consts = ctx.enter_context(tc.tile_pool(name="consts", bufs=1))
        xst = ctx.enter_context(tc.tile_pool(name="xst", bufs=4))
        spl = ctx.enter_context(tc.tile_pool(name="spl", bufs=8))
        a1p = ctx.enter_context(tc.tile_pool(name="a1p", bufs=8))
        yp = ctx.enter_context(tc.tile_pool(name="yp", bufs=2))
        up = ctx.enter_context(tc.tile_pool(name="up", bufs=6))
        pA = ctx.enter_context(tc.tile_pool(name="pA", bufs=4, space="PSUM"))
        pS = ctx.enter_context(tc.tile_pool(name="pS", bufs=4, space="PSUM"))

        # w1/wsc/bnc load first (needed by group 0); w2's DMA is emitted after
        # group 0's split section so it doesn't block startup.
        w1_sb = consts.tile([128, 9 * NCI1, COUT], F16)
        wsc_sb = consts.tile([128, NCI1, COUT], F16)
        bnc_sb = consts.tile([128, 5, NCT], F32)
        w2_sb = consts.tile([128, 9 * NCI2, COUT], BF)

        def bn_ap(i, c):
            return bnc_sb[:, i, c:c + 1]

        for g in range(NG):
            bsl = slice(g * G, (g + 1) * G)
            # ---- load x and split into 2 exact fp16 terms (in-place resid) --
            parts = []  # parts[ci] = (hi, lo)
            for ci in range(NCI1):
                xt = xst.tile([128, G, 841], F32, tag="xst", name=f"x_{g}_{ci}")
                nc.sync.dma_start(xt[:], xp[ci][:, bsl])
                if g == 0 and ci == 0:
                    nc.sync.dma_start(w1_sb[:], w1[:])
                    nc.sync.dma_start(wsc_sb[:], wsc[:])
                    nc.sync.dma_start(bnc_sb[:], bnc[:])
                hi = spl.tile([128, G, 841], F16, tag="spl", name=f"hi_{g}_{ci}")
                nc.scalar.copy(hi[:], xt[:])
                nc.vector.tensor_sub(xt[:], xt[:], hi[:])
                lo = spl.tile([128, G, 841], F16, tag="spl", name=f"lo_{g}_{ci}")
                nc.vector.tensor_copy(lo[:], xt[:])
                parts.append((hi, lo))
            if g == 0:
                nc.sync.dma_start(w2_sb[:], w2[:])

            # ---- conv1: taps stream one contiguous plane window each ----
            # psum [128, G, 14, 16]; valid cols 1..14 (col 0/15 garbage)
            p1 = []
            for c in range(NCT):
                pt = pA.tile([128, G, 14, 16], F32, tag="pA", name=f"p1_{g}_{c}")
                idx, last = 0, NCI1 * 9 * NSPL - 1
                for s in range(NSPL):
                    for ci in range(NCI1):
                        for t in range(9):
                            kh, kw = divmod(t, 3)
                            ph, pw = kh & 1, kw & 1
                            w_pl = PL_W[pw]
                            dh = 1 if kh == 2 else 0
                            off = PL_OFF[(ph, pw)] + dh * w_pl
                            c0 = 0 if kw == 2 else 1
                            w_ap = w1_sb[:, t * NCI1 + ci, c * 128:(c + 1) * 128]
                            rhs = parts[ci][s][:, :, off:off + 14 * w_pl]
                            nc.tensor.matmul(pt[:, :, :, c0:c0 + w_pl], w_ap, rhs,
                                             start=(idx == 0), stop=(idx == last))
                            idx += 1
                p1.append(pt)

            # ---- shortcut: 1x1 stride-2 conv = odd/odd parity plane ----
            psc = []
            for c in range(NCT):
                pt = pS.tile([128, NPG], F32, tag="pS", name=f"psc_{g}_{c}")
                idx, last = 0, NCI1 * NSPL - 1
                for s in range(NSPL):
                    for ci in range(NCI1):
                        w_ap = wsc_sb[:, ci, c * 128:(c + 1) * 128]
                        rhs = parts[ci][s][:, :, 645:841]
                        nc.tensor.matmul(pt[:], w_ap, rhs,
                                         start=(idx == 0), stop=(idx == last))
                        idx += 1
                psc.append(pt)

            # ---- a1 = sign(bn1(conv1)), unpadded bf16 [G,14,14] ----
            a1 = []
            for c in range(NCT):
                at = a1p.tile([128, G, OH, OH], BF, tag="a1", name=f"a1_{g}_{c}")
                nc.scalar.activation(at[:], p1[c][:, :, :, 1:15],
                                     SIGN, bias=bn_ap(1, c), scale=bn_ap(0, c))
                a1.append(at)

            # ---- conv2: whole-a1 streams into shifted 16x16 psum window ----
            # psum [128, G, 16, 16]; valid [1:15, 1:15]
            p2 = []
            for c in range(NCT):
                pt = pA.tile([128, G, 16, 16], F32, tag="pA", name=f"p2_{g}_{c}")
                idx, last = 0, NCI2 * 9 - 1
                for ci in range(NCI2):
                    for t in range(9):
                        kh, kw = divmod(t, 3)
                        w_ap = w2_sb[:, t * NCI2 + ci, c * 128:(c + 1) * 128]
                        out = pt[:, :, 2 - kh:16 - kh, 2 - kw:16 - kw]
                        nc.tensor.matmul(out, w_ap, a1[ci][:],
                                         start=(idx == 0), stop=(idx == last))
                        idx += 1
                p2.append(pt)

            # ---- y = sign(scale2*p2 + shift2 + scalesc*psc + shiftsc) ----
            yt = yp.tile([128, NCT, NPG], F32, tag="y", name=f"y_{g}")
            for c in range(NCT):
                # w2t = scale2*p2 + (shift2+shiftsc): ACT handles the 4D
                # strided psum window; stt only takes 2D/3D operands.
                wt = up.tile([128, NPG], F32, tag="u", name=f"u_{g}_{c}")
                nc.scalar.activation(
                    wt[:].rearrange("p (b h w) -> p b h w", b=G, h=OH, w=OH),
                    p2[c][:, :, 1:15, 1:15], IDENT,
                    bias=bn_ap(3, c), scale=bn_ap(2, c))
                vt = up.tile([128, NPG], F32, tag="v", bufs=3, name=f"v_{g}_{c}")
                nc.vector.scalar_tensor_tensor(
                    vt[:], psc[c][:], bn_ap(4, c), wt[:],
                    op0=mybir.AluOpType.mult, op1=mybir.AluOpType.add)
                nc.scalar.activation(yt[:, c, :], vt[:], SIGN)
            nc.sync.dma_start(
                y[:, :, bsl].rearrange("p c b x -> p c (b x)"), yt[:])

    nc.compile()
    return nc


def _prep_consts(w1, w2, wsc, g1, b1, m1, v1, g2, b2, m2, v2, gsc, bsc, msc, vsc):
    def sgn_w(w, dt):
        return np.where(w >= 0, np.float32(1.0), np.float32(-1.0)).astype(dt)

    # lhsT layouts: [cin_part(128), tap*NCI+ci, cout]
    w1s = sgn_w(w1, np.float16)  # [COUT, CIN, 3, 3]
    a1w = np.empty((128, 9 * NCI1, COUT), np.float16)
    for t in range(9):
        kh, kw = divmod(t, 3)
        for ci in range(NCI1):
            a1w[:, t * NCI1 + ci, :] = w1s[:, ci * 128:(ci + 1) * 128, kh, kw].T
    w2s = sgn_w(w2, bf16)
    a2w = np.empty((128, 9 * NCI2, COUT), bf16)
    for t in range(9):
        kh, kw = divmod(t, 3)
        for ci in range(NCI2):
            a2w[:, t * NCI2 + ci, :] = w2s[:, ci * 128:(ci + 1) * 128, kh, kw].T
    wscs = sgn_w(wsc, np.float16)
    asw = np.empty((128, NCI1, COUT), np.float16)
    for ci in range(NCI1):
        asw[:, ci, :] = wscs[:, ci * 128:(ci + 1) * 128, 0, 0].T

    def bn_affine(g, b, m, v):
        scale = (g / np.sqrt(v + EPS)).astype(np.float32)
        shift = (b - m * g / np.sqrt(v + EPS)).astype(np.float32)
        return scale, shift

    sc1, sh1 = bn_affine(g1, b1, m1, v1)
    sc2, sh2 = bn_affine(g2, b2, m2, v2)
    scs, shs = bn_affine(gsc, bsc, msc, vsc)
    bnc = np.empty((128, 5, NCT), np.float32)
    for c in range(NCT):
        cs = slice(c * 128, (c + 1) * 128)
        bnc[:, 0, c] = sc1[cs]
        bnc[:, 1, c] = sh1[cs]
        bnc[:, 2, c] = sc2[cs]
        bnc[:, 3, c] = (sh2 + shs)[cs]
        bnc[:, 4, c] = scs[cs]
    return a1w, a2w, asw, bnc


def kernel(x, w1, g1, b1, m1, v1, w2, g2, b2, m2, v2, wsc, gsc, bsc, msc, vsc,
           _trace=False):
    x = np.ascontiguousarray(x, np.float32)
    a1w, a2w, asw, bnc = _prep_consts(
        np.asarray(w1, np.float32), np.asarray(w2, np.float32),
        np.asarray(wsc, np.float32),
        *[np.asarray(t, np.float32) for t in (g1, b1, m1, v1)],
        *[np.asarray(t, np.float32) for t in (g2, b2, m2, v2)],
        *[np.asarray(t, np.float32) for t in (gsc, bsc, msc, vsc)])

    # padded, channel-major x repacked as concatenated stride-2 parity planes
    xpad = np.zeros((CIN, B, H + 1, H + 1), np.float32)
    xpad[:, :, 1:, 1:] = x.transpose(1, 0, 2, 3)
    xflat = np.concatenate(
        [xpad[:, :, ph::2, pw::2].reshape(CIN, B, -1)
         for ph in (0, 1) for pw in (0, 1)], axis=2)  # [CIN, B, 841]

    if "nc" not in _prog_cache:
        _prog_cache["nc"] = _build_program()
    nc = _prog_cache["nc"]

    in_maps = []
    for k in range(N_CORES):
        m = {"w1t": a1w, "w2t": a2w, "wsct": asw, "bnc": bnc}
        for ci in range(NCI1):
            m[f"xp{ci}"] = np.ascontiguousarray(
                xflat[ci * 128:(ci + 1) * 128, k * BPC:(k + 1) * BPC])
        in_maps.append(m)

    res = run_bass_kernel_spmd(nc, in_maps, core_ids=list(range(N_CORES)),
                               trace=_trace)

    # y dram: [128, NCT, BPC, 196] per core -> [B, COUT, 14, 14]
    out = np.empty((B, COUT, OH, OH), np.float32)
    for k in range(N_CORES):
        yk = res.results[k]["y"]  # [128, 4, 16, 196]
        out[k * BPC:(k + 1) * BPC] = (
            yk.transpose(2, 1, 0, 3).reshape(BPC, COUT, OH, OH))
    if _trace:
        kernel.last_results = res
    return out


# revision 14
# speedup vs baseline: 1.3731x; 1.0033x over previous
"""Binarized ResNet BasicBlock (conv1 3x3/s2 + BN + sign, conv2 3x3 + BN,
1x1/s2 shortcut conv + BN, add, sign) as a Bass/Tile kernel on 8 TRN2 cores.

Strategy:
- Data-parallel over batch: 16 images per core, weights/BN params replicated.
- Binarized weights are exactly +-1 in fp16/bf16. x is split into 2 fp16
  terms (hi = fp16(x), lo = fp16(x - hi)) whose products with +-1 weights
  are exact (PE handles fp16 subnormals exactly; verified on HW), so conv1
  and the shortcut accumulate x to ~2^-22 relative in fp32 PSUM. conv2's
  inputs are +-1 bf16 (exact); its accumulation is exact integer arithmetic.
- sign(clip(bn(z))) == sign(bn(z)): fused into one Sign activation with
  per-channel scale/bias APs.
- Consecutive matmuls never reuse a stationary weight tile (same-weight
  back-to-back serializes LDWEIGHTS; rotating weights pipelines it away).
- Conv taps stream fully-contiguous rhs and land at tap-dependent PSUM
  offsets (per-element has_written gives overwrite-then-accumulate), instead
  of strided 28-byte windows: x is packed as stride-2 parity planes so every
  conv1 tap reads one contiguous span; conv2 streams whole unpadded a1
  tiles into a 16x16 PSUM window with garbage borders.
"""

import numpy as np
import ml_dtypes
from contextlib import ExitStack

import concourse.bass as bass
import concourse.tile as tile
from concourse import mybir, bacc
from concourse.bass_utils import run_bass_kernel_spmd

bf16 = ml_dtypes.bfloat16
F32 = mybir.dt.float32
BF = mybir.dt.bfloat16
F16 = mybir.dt.float16
SIGN = mybir.ActivationFunctionType.Sign
IDENT = mybir.ActivationFunctionType.Identity

N_CORES = 8
B, CIN, COUT, H = 128, 256, 512, 28
OH = 14                      # output spatial
BPC = B // N_CORES           # images per core
G = 2                        # images per matmul group
NG = BPC // G                # groups per core
NPG = G * OH * OH            # 392 valid pixels per group
NCT = COUT // 128            # cout tiles (4)
NCI1 = CIN // 128            # cin tiles for conv1/shortcut (2)
NCI2 = COUT // 128           # cin tiles for conv2 (4)
NSPL = 2                     # split terms for x (fp16 hi/lo)
EPS = np.float32(1e-5)

# parity-plane packing of the 29x29 zero-padded input (pad at index 0):
# plane (ph, pw) = xpad[2i+ph, 2j+pw]; heights/widths 15 or 14.
PL_H = {0: 15, 1: 14}
PL_W = {0: 15, 1: 14}
PL_OFF = {(0, 0): 0, (0, 1): 225, (1, 0): 435, (1, 1): 645}

_prog_cache = {}


def _build_program():
    nc = bacc.Bacc("TRN2", debug=False)

    xp = [nc.dram_tensor(f"xp{ci}", [128, BPC, 841], F32,
                         kind="ExternalInput").ap() for ci in range(NCI1)]
    w1 = nc.dram_tensor("w1t", [128, 9 * NCI1, COUT], F16, kind="ExternalInput").ap()
    w2 = nc.dram_tensor("w2t", [128, 9 * NCI2, COUT], BF, kind="ExternalInput").ap()
    wsc = nc.dram_tensor("wsct", [128, NCI1, COUT], F16, kind="ExternalInput").ap()
    bnc = nc.dram_tensor("bnc", [128, 5, NCT], F32, kind="ExternalInput").ap()
    y = nc.dram_tensor("y", [128, NCT, BPC, OH * OH], F32,
                       kind="ExternalOutput").ap()

    with tile.TileContext(nc) as tc, ExitStack() as ctx:
        consts = ctx.enter_context(tc.tile_pool(name="consts", bufs=1))
        xst = ctx.enter_context(tc.tile_pool(name="xst", bufs=6))
        spl = ctx.enter_context(tc.tile_pool(name="spl", bufs=12))
        a1p = ctx.enter_context(tc.tile_pool(name="a1p", bufs=12))
        yp = ctx.enter_context(tc.tile_pool(name="yp", bufs=3))
        up = ctx.enter_context(tc.tile_pool(name="up", bufs=6))
        pA = ctx.enter_context(tc.tile_pool(name="pA", bufs=4, space="PSUM"))
        pS = ctx.enter_context(tc.tile_pool(name="pS", bufs=4, space="PSUM"))

        # w1/wsc/bnc load first (needed by group 0); w2's DMA is emitted after
        # group 0's split section so it doesn't block startup.
        w1_sb = consts.tile([128, 9 * NCI1, COUT], F16)
        wsc_sb = consts.tile([128, NCI1, COUT], F16)
        bnc_sb = consts.tile([128, 5, NCT], F32)
        w2_sb = consts.tile([128, 9 * NCI2, COUT], BF)

        def bn_ap(i, c):
            return bnc_sb[:, i, c:c + 1]

        # group schedule: two single-image groups first (shorter fill for the
        # PE pipeline at startup), then 2-image groups.
        sched = [(0, 1), (1, 1)] + [(b0, G) for b0 in range(G, BPC, G)]
        for gi, (b0, gs) in enumerate(sched):
            bsl = slice(b0, b0 + gs)
            npg = gs * OH * OH
            # ---- load x and split into 2 exact fp16 terms (in-place resid) --
            parts = []  # parts[ci] = (hi, lo)
            for ci in range(NCI1):
                xt = xst.tile([128, G, 841], F32, tag="xst", name=f"x_{gi}_{ci}")
                nc.sync.dma_start(xt[:, 0:gs], xp[ci][:, bsl])
                if gi == 0 and ci == 0:
                    nc.sync.dma_start(w1_sb[:], w1[:])
                    nc.sync.dma_start(wsc_sb[:], wsc[:])
                    nc.sync.dma_start(bnc_sb[:], bnc[:])
                hi = spl.tile([128, G, 841], F16, tag="spl", name=f"hi_{gi}_{ci}")
                nc.scalar.copy(hi[:, 0:gs], xt[:, 0:gs])
                nc.vector.tensor_sub(xt[:, 0:gs], xt[:, 0:gs], hi[:, 0:gs])
                lo = spl.tile([128, G, 841], F16, tag="spl", name=f"lo_{gi}_{ci}")
                nc.vector.tensor_copy(lo[:, 0:gs], xt[:, 0:gs])
                parts.append((hi, lo))
            if gi == 0:
                nc.sync.dma_start(w2_sb[:], w2[:])

            # ---- conv1 + interleaved shortcut matmuls ----
            # conv1 psum [128, gs, 14, 16]; valid cols 1..14
            p1, psc = [], []
            for c in range(NCT):
                pt = pA.tile([128, gs, 14, 16], F32, tag="pA", name=f"p1_{gi}_{c}")
                idx, last = 0, NCI1 * 9 * NSPL - 1
                for s in range(NSPL):
                    for ci in range(NCI1):
                        for t in range(9):
                            kh, kw = divmod(t, 3)
                            ph, pw = kh & 1, kw & 1
                            w_pl = PL_W[pw]
                            dh = 1 if kh == 2 else 0
                            off = PL_OFF[(ph, pw)] + dh * w_pl
                            c0 = 0 if kw == 2 else 1
                            w_ap = w1_sb[:, t * NCI1 + ci, c * 128:(c + 1) * 128]
                            rhs = parts[ci][s][:, 0:gs, off:off + 14 * w_pl]
                            nc.tensor.matmul(pt[:, :, :, c0:c0 + w_pl], w_ap, rhs,
                                             start=(idx == 0), stop=(idx == last))
                            idx += 1
                p1.append(pt)
                # shortcut for this cout tile: odd/odd parity plane
                st = pS.tile([128, NPG], F32, tag="pS", name=f"psc_{gi}_{c}")
                idx, last = 0, NCI1 * NSPL - 1
                for s in range(NSPL):
                    for ci in range(NCI1):
                        w_ap = wsc_sb[:, ci, c * 128:(c + 1) * 128]
                        rhs = parts[ci][s][:, 0:gs, 645:841]
                        nc.tensor.matmul(st[:, 0:npg], w_ap, rhs,
                                         start=(idx == 0), stop=(idx == last))
                        idx += 1
                psc.append(st)

            # ---- a1 = sign(bn1(conv1)), unpadded bf16 [gs,14,14] ----
            a1 = []
            for c in range(NCT):
                at = a1p.tile([128, G, OH, OH], BF, tag="a1", name=f"a1_{gi}_{c}")
                nc.scalar.activation(at[:, 0:gs], p1[c][:, :, :, 1:15],
                                     SIGN, bias=bn_ap(1, c), scale=bn_ap(0, c))
                a1.append(at)

            # ---- conv2: whole-a1 streams into shifted 16x16 psum window ----
            # psum [128, gs, 16, 16]; valid [1:15, 1:15]
            p2 = []
            for c in range(NCT):
                pt = pA.tile([128, gs, 16, 16], F32, tag="pA", name=f"p2_{gi}_{c}")
                idx, last = 0, NCI2 * 9 - 1
                for ci in range(NCI2):
                    for t in range(9):
                        kh, kw = divmod(t, 3)
                        w_ap = w2_sb[:, t * NCI2 + ci, c * 128:(c + 1) * 128]
                        out = pt[:, :, 2 - kh:16 - kh, 2 - kw:16 - kw]
                        nc.tensor.matmul(out, w_ap, a1[ci][:, 0:gs],
                                         start=(idx == 0), stop=(idx == last))
                        idx += 1
                p2.append(pt)

            # ---- y = sign(scale2*p2 + shift2 + scalesc*psc + shiftsc) ----
            yt = yp.tile([128, NCT, NPG], F32, tag="y", name=f"y_{gi}")
            for c in range(NCT):
                # wt = scale2*p2 + (shift2+shiftsc): ACT handles the 4D
                # strided psum window; stt only takes 2D/3D operands.
                wt = up.tile([128, NPG], F32, tag="u", name=f"u_{gi}_{c}")
                nc.scalar.activation(
                    wt[:, 0:npg].rearrange("p (b h w) -> p b h w",
                                           b=gs, h=OH, w=OH),
                    p2[c][:, :, 1:15, 1:15], IDENT,
                    bias=bn_ap(3, c), scale=bn_ap(2, c))
                vt = up.tile([128, NPG], F32, tag="v", bufs=3, name=f"v_{gi}_{c}")
                nc.vector.scalar_tensor_tensor(
                    vt[:, 0:npg], psc[c][:, 0:npg], bn_ap(4, c), wt[:, 0:npg],
                    op0=mybir.AluOpType.mult, op1=mybir.AluOpType.add)
                nc.scalar.activation(yt[:, c, 0:npg], vt[:, 0:npg], SIGN)
            nc.sync.dma_start(
                y[:, :, bsl].rearrange("p c b x -> p c (b x)"),
                yt[:, :, 0:npg])

    nc.compile()
    return nc


def _prep_consts(w1, w2, wsc, g1, b1, m1, v1, g2, b2, m2, v2, gsc, bsc, msc, vsc):
    def sgn_w(w, dt):
        return np.where(w >= 0, np.float32(1.0), np.float32(-1.0)).astype(dt)

    # lhsT layouts: [cin_part(128), tap*NCI+ci, cout]
    w1s = sgn_w(w1, np.float16)  # [COUT, CIN, 3, 3]
    a1w = np.empty((128, 9 * NCI1, COUT), np.float16)
    for t in range(9):
        kh, kw = divmod(t, 3)
        for ci in range(NCI1):
            a1w[:, t * NCI1 + ci, :] = w1s[:, ci * 128:(ci + 1) * 128, kh, kw].T
    w2s = sgn_w(w2, bf16)
    a2w = np.empty((128, 9 * NCI2, COUT), bf16)
    for t in range(9):
        kh, kw = divmod(t, 3)
        for ci in range(NCI2):
            a2w[:, t * NCI2 + ci, :] = w2s[:, ci * 128:(ci + 1) * 128, kh, kw].T
    wscs = sgn_w(wsc, np.float16)
    asw = np.empty((128, NCI1, COUT), np.float16)
    for ci in range(NCI1):
        asw[:, ci, :] = wscs[:, ci * 128:(ci + 1) * 128, 0, 0].T

    def bn_affine(g, b, m, v):
        scale = (g / np.sqrt(v + EPS)).astype(np.float32)
        shift = (b - m * g / np.sqrt(v + EPS)).astype(np.float32)
        return scale, shift

    sc1, sh1 = bn_affine(g1, b1, m1, v1)
    sc2, sh2 = bn_affine(g2, b2, m2, v2)
    scs, shs = bn_affine(gsc, bsc, msc, vsc)
    bnc = np.empty((128, 5, NCT), np.float32)
    for c in range(NCT):
        cs = slice(c * 128, (c + 1) * 128)
        bnc[:, 0, c] = sc1[cs]
        bnc[:, 1, c] = sh1[cs]
        bnc[:, 2, c] = sc2[cs]
        bnc[:, 3, c] = (sh2 + shs)[cs]
        bnc[:, 4, c] = scs[cs]
    return a1w, a2w, asw, bnc


def kernel(x, w1, g1, b1, m1, v1, w2, g2, b2, m2, v2, wsc, gsc, bsc, msc, vsc,
           _trace=False):
    x = np.ascontiguousarray(x, np.float32)
    a1w, a2w, asw, bnc = _prep_consts(
        np.asarray(w1, np.float32), np.asarray(w2, np.float32),
        np.asarray(wsc, np.float32),
        *[np.asarray(t, np.float32) for t in (g1, b1, m1, v1)],
        *[np.asarray(t, np.float32) for t in (g2, b2, m2, v2)],
        *[np.asarray(t, np.float32) for t in (gsc, bsc, msc, vsc)])

    # padded, channel-major x repacked as concatenated stride-2 parity planes
    xpad = np.zeros((CIN, B, H + 1, H + 1), np.float32)
    xpad[:, :, 1:, 1:] = x.transpose(1, 0, 2, 3)
    xflat = np.concatenate(
        [xpad[:, :, ph::2, pw::2].reshape(CIN, B, -1)
         for ph in (0, 1) for pw in (0, 1)], axis=2)  # [CIN, B, 841]

    if "nc" not in _prog_cache:
        _prog_cache["nc"] = _build_program()
    nc = _prog_cache["nc"]

    in_maps = []
    for k in range(N_CORES):
        m = {"w1t": a1w, "w2t": a2w, "wsct": asw, "bnc": bnc}
        for ci in range(NCI1):
            m[f"xp{ci}"] = np.ascontiguousarray(
                xflat[ci * 128:(ci + 1) * 128, k * BPC:(k + 1) * BPC])
        in_maps.append(m)

    res = run_bass_kernel_spmd(nc, in_maps, core_ids=list(range(N_CORES)),
                               trace=_trace)

    # y dram: [128, NCT, BPC, 196] per core -> [B, COUT, 14, 14]
    out = np.empty((B, COUT, OH, OH), np.float32)
    for k in range(N_CORES):
        yk = res.results[k]["y"]  # [128, 4, 16, 196]
        out[k * BPC:(k + 1) * BPC] = (
            yk.transpose(2, 1, 0, 3).reshape(BPC, COUT, OH, OH))
    if _trace:
        kernel.last_results = res
    return out
